# revision 1
# baseline (speedup 1.0000x reference)
"""TRN2 Bass kernel for nn_LocalAttention (B=4, T=2048, C=1024, window=16).

Sharding: 8 cores = (batch b, row-half h). Each core computes K^T/V for its
whole batch (duplicated across the 2 cores of a batch) and attention +
projections for its own 1024 rows (two 512-row chunks; h=0 gets global
chunks {0,3}, h=1 gets {1,2}; slot 0 = denser chunk).

All matmuls run in fp32r (TF32-like, ~1.5e-4 rel err, 4x fp32 speed). Raw
fp32 bytes are declared as fp32r at the DRAM boundary - the PE rounds
internally (validated: identical error to explicit cast-DMA).

Orientation trick: host passes X^T and W^T so every matmul is natural:
  K^T = (Wk^T)^T @ X^T        [C, T]     (DRAM scratch)
  V   = (X^T)^T @ Wv^T        [T, C]     (DRAM scratch)
  Q^T = (Wq^T)^T @ X_own^T    [C, 1024]  (SBUF resident)
  S^T = (K^T_blk)^T @ Q^T_chunk  -> [keys, rows]; softmax-over-keys is a
        partition reduction done by a ones-vector matmul, and E^T feeds
  Y^T = V_blk^T @ E^T            [C, rows]
  Z^T = (Wo^T)^T @ Y^T           [C, rows]

Sparsity: mask keeps j >= i - 16 (reverse-causal), so each 512-row chunk's
kept key-block set is a SUFFIX {b..15}; processing key blocks in descending
order (position p -> block 15-p) makes every kept set a static PREFIX.
Chunk slot 0 runs 16 positions, slot 1 runs 9 - uniform across cores, the
data-driven is_ge mask zeroes over-included blocks. Mask applied
multiplicatively post-exp (scores are O(6), no overflow without max-sub).
"""
import numpy as np

import concourse.bass as bass
import concourse.mybir as mybir
import concourse.tile as tile
from concourse import bacc
from concourse import bass_utils

N_CORES = 8
B, T, C = 4, 2048, 1024
WINDOW = 16
TOWN = T // 2          # own rows per core
CHUNK = 512            # rows per processing chunk
NCHUNK = TOWN // CHUNK  # 2
CI = C // 128          # 8 contraction blocks
CO = C // 128          # 8 output blocks
KB = T // 128          # 16 key blocks
TCH = T // CHUNK       # 4 t-chunks in phase A
SLOT_KBS = (16, 9)     # key-block positions per chunk slot (descending order)
F32 = mybir.dt.float32
F32R = mybir.dt.float32r

_NC_CACHE = {}


def build():
    if "nc" in _NC_CACHE:
        return _NC_CACHE["nc"]
    nc = bacc.Bacc("TRN2", target_bir_lowering=False, debug=False,
                   num_devices=N_CORES)
    xt = nc.dram_tensor("xt", [C, T], F32R, kind="ExternalInput").ap()
    xtq = nc.dram_tensor("xtq", [C, TOWN], F32R, kind="ExternalInput").ap()
    wqt = nc.dram_tensor("wqt", [C, C], F32R, kind="ExternalInput").ap()
    wkt = nc.dram_tensor("wkt", [C, C], F32R, kind="ExternalInput").ap()
    wvt = nc.dram_tensor("wvt", [C, C], F32R, kind="ExternalInput").ap()
    wot = nc.dram_tensor("wot", [C, C], F32R, kind="ExternalInput").ap()
    keyidx16 = nc.dram_tensor("keyidx16", [128, KB], F32, kind="ExternalInput").ap()
    rowidxb = nc.dram_tensor("rowidxb", [128, TOWN], F32, kind="ExternalInput").ap()
    zt = nc.dram_tensor("zt", [C, TOWN], F32, kind="ExternalOutput").ap()

    xt3 = xt.rearrange("(ko ki) t -> ki ko t", ki=128)
    xtq3 = xtq.rearrange("(ko ki) t -> ki ko t", ki=128)
    w3 = {w.tensor.name: w.rearrange("(ko ki) c -> ki ko c", ki=128)
          for w in (wqt, wkt, wvt, wot)}

    inv_sqrt_c = float(1.0 / np.sqrt(C))

    with tile.TileContext(nc) as tc:
        with tc.tile_pool(name="res", bufs=1) as res, \
             tc.tile_pool(name="dram", bufs=1, space="DRAM") as dram:
            kt_d = dram.tile([128, CI, T], F32R)      # K^T  [ki, ko, t]
            v_d = dram.tile([128, KB, C], F32R)       # V    [ki, ko, c]
            qt_sb = res.tile([128, CI, TOWN], F32R, tag="qt")  # Q^T resident
            wo_sb = res.tile([128, CI, C], F32R, tag="wo")
            ki16_sb = res.tile([128, KB], F32, tag="ki16")
            nc.gpsimd.dma_start(ki16_sb[:], keyidx16[:])
            ones_row_f32 = res.tile([1, 128], F32, tag="onesrf")
            nc.vector.memset(ones_row_f32[:], 1.0)
            ones_1x128 = res.tile([1, 128], F32R, tag="o1")
            nc.vector.tensor_copy(ones_1x128[:], ones_row_f32[:])
            ones_col_f32 = res.tile([128, 1], F32, tag="onescf")
            nc.vector.memset(ones_col_f32[:], 1.0)
            ones_128x1 = res.tile([128, 1], F32R, tag="o2")
            nc.vector.tensor_copy(ones_128x1[:], ones_col_f32[:])

            # ================= Phase A: projections =========================
            with tc.tile_pool(name="wts", bufs=1) as wts, \
                 tc.tile_pool(name="xa", bufs=2) as xa, \
                 tc.tile_pool(name="stg", bufs=3) as stg, \
                 tc.tile_pool(name="ps_k", bufs=3, space="PSUM") as ps_k, \
                 tc.tile_pool(name="ps_v", bufs=2, space="PSUM") as ps_v, \
                 tc.tile_pool(name="ps_q", bufs=2, space="PSUM") as ps_q:
                wk_sb = wts.tile([128, CI, C], F32R, tag="wk")
                wv_sb = wts.tile([128, CI, C], F32R, tag="wv")
                wq_sb = wts.tile([128, CI, C], F32R, tag="wq")
                # first xt chunk before anything else on the sync queue
                xt_sbs = []
                xt_sb0 = xa.tile([128, CI, CHUNK], F32R, tag="xa")
                nc.sync.dma_start(xt_sb0[:], xt3[:, :, (TCH - 1) * CHUNK:TCH * CHUNK])
                for co in range(CO):  # per-column loads: co=0 unblocks MMs
                    nc.sync.dma_start(wk_sb[:, :, co * 128:(co + 1) * 128],
                                      w3["wkt"][:, :, co * 128:(co + 1) * 128])
                for ci in range(CI):
                    nc.scalar.dma_start(wv_sb[:, ci, :], w3["wvt"][:, ci, :])
                for ci in range(CI):
                    nc.scalar.dma_start(wq_sb[:, ci, :], w3["wqt"][:, ci, :])

                for tch in reversed(range(TCH)):
                    if tch == TCH - 1:
                        xt_sb = xt_sb0
                    else:
                        xt_sb = xa.tile([128, CI, CHUNK], F32R, tag="xa")
                        nc.sync.dma_start(
                            xt_sb[:], xt3[:, :, tch * CHUNK:(tch + 1) * CHUNK])
                    # K^T [cout, t]
                    for co in range(CO):
                        kps = ps_k.tile([128, CHUNK], F32, tag="kps")
                        for ci in range(CI):
                            nc.tensor.matmul(
                                kps[:], wk_sb[:, ci, co * 128:(co + 1) * 128],
                                xt_sb[:, ci, :], start=(ci == 0), stop=(ci == CI - 1))
                        kstage = stg.tile([128, CHUNK], F32R, tag="kstage")
                        nc.vector.tensor_copy(kstage[:], kps[:])
                        nc.sync.dma_start(
                            kt_d[:, co, tch * CHUNK:(tch + 1) * CHUNK], kstage[:])
                    # V [t, cout]
                    for tb in range(CHUNK // 128):
                        for half in range(2):
                            vps = ps_v.tile([128, 512], F32, tag="vps")
                            for ci in range(CI):
                                nc.tensor.matmul(
                                    vps[:], xt_sb[:, ci, tb * 128:(tb + 1) * 128],
                                    wv_sb[:, ci, half * 512:(half + 1) * 512],
                                    start=(ci == 0), stop=(ci == CI - 1))
                            vstage = stg.tile([128, 512], F32R, tag="vstage")
                            nc.vector.tensor_copy(vstage[:], vps[:])
                            nc.scalar.dma_start(
                                v_d[:, tch * (CHUNK // 128) + tb,
                                    half * 512:(half + 1) * 512], vstage[:])

                for qch in range(TOWN // CHUNK):
                    xq_sb = xa.tile([128, CI, CHUNK], F32R, tag="xa")
                    nc.sync.dma_start(
                        xq_sb[:], xtq3[:, :, qch * CHUNK:(qch + 1) * CHUNK])
                    for co in range(CO):
                        qps = ps_q.tile([128, CHUNK], F32, tag="qps")
                        for ci in range(CI):
                            nc.tensor.matmul(
                                qps[:], wq_sb[:, ci, co * 128:(co + 1) * 128],
                                xq_sb[:, ci, :], start=(ci == 0), stop=(ci == CI - 1))
                        nc.vector.tensor_copy(
                            qt_sb[:, co, qch * CHUNK:(qch + 1) * CHUNK], qps[:])

            # wo on the gpsimd (SWDGE) queue: latency-insensitive, keeps the
            # HW-DGE queues free for phase-B kt/v streams
            for ci in range(CI):
                nc.gpsimd.dma_start(wo_sb[:, ci, :], w3["wot"][:, ci, :])

            # ================= Phase B: attention + out-proj ================
            with tc.tile_pool(name="et", bufs=1) as etp, \
                 tc.tile_pool(name="ktb", bufs=4) as ktb_p, \
                 tc.tile_pool(name="vco", bufs=3) as vsp, \
                 tc.tile_pool(name="ysb", bufs=2) as ysb_p, \
                 tc.tile_pool(name="wb", bufs=2) as wb, \
                 tc.tile_pool(name="zst", bufs=3) as zstp, \
                 tc.tile_pool(name="ps_s", bufs=3, space="PSUM") as ps_s, \
                 tc.tile_pool(name="ps_sh", bufs=1, space="PSUM") as ps_sh, \
                 tc.tile_pool(name="ps_y", bufs=2, space="PSUM") as ps_y, \
                 tc.tile_pool(name="ps_z", bufs=2, space="PSUM") as ps_z:
                for ch in range(NCHUNK):
                    nkb = SLOT_KBS[ch]
                    rsl = slice(ch * CHUNK, (ch + 1) * CHUNK)
                    ri_b = wb.tile([128, CHUNK], F32, tag="rib")
                    nc.sync.dma_start(ri_b[:], rowidxb[:, rsl])

                    et = etp.tile([128, KB, CHUNK], F32R, tag="et")
                    # --- sweep 1a: scores + exp + mask (descending kb) ---
                    for p in range(nkb):
                        kb = KB - 1 - p
                        kt_b = ktb_p.tile([128, CI, 128], F32R, tag="ktb")
                        nc.scalar.dma_start(
                            kt_b[:], kt_d[:, :, kb * 128:(kb + 1) * 128])
                        sps = ps_s.tile([128, CHUNK], F32, tag="sps")
                        for ci in range(CI):
                            nc.tensor.matmul(
                                sps[:], kt_b[:, ci, :], qt_sb[:, ci, rsl],
                                start=(ci == 0), stop=(ci == CI - 1))
                        nc.scalar.activation(et[:, p, :], sps[:],
                                             mybir.ActivationFunctionType.Exp,
                                             scale=inv_sqrt_c)
                        mask = wb.tile([128, CHUNK], F32, tag="mask")
                        nc.vector.tensor_tensor(
                            mask[:], ki16_sb[:, kb:kb + 1].to_broadcast((128, CHUNK)),
                            ri_b[:], mybir.AluOpType.is_ge)
                        nc.vector.tensor_tensor(et[:, p, :], et[:, p, :], mask[:],
                                                mybir.AluOpType.mult)
                    # --- sweep 1b: key-sums via ones matmul ---
                    sums_ps = ps_sh.tile([1, CHUNK], F32, tag="shared")
                    for p in range(nkb):
                        nc.tensor.matmul(sums_ps[:], ones_128x1[:], et[:, p, :],
                                         start=(p == 0), stop=(p == nkb - 1))
                    recip = wb.tile([1, CHUNK], F32R, tag="recip")
                    with nc.allow_low_precision(reason="fp32r normalizer broadcast"):
                        nc.vector.reciprocal(recip[:], sums_ps[:])
                    rb_ps = ps_sh.tile([128, CHUNK], F32, tag="shared")
                    nc.tensor.matmul(rb_ps[:], ones_1x128[:], recip[:],
                                     start=True, stop=True)
                    rb_sb = wb.tile([128, CHUNK], F32, tag="rbsb")
                    nc.vector.tensor_copy(rb_sb[:], rb_ps[:])

                    # --- sweep 2: Y^T = V^T @ E^T per cout block ---
                    y_sb = ysb_p.tile([128, CO, CHUNK], F32R, tag="ysb")
                    for co in range(CO):
                        v_co = vsp.tile([128, KB, 128], F32R, tag="vco")
                        nc.sync.dma_start(
                            v_co[:, :nkb, :],
                            v_d[:, KB - nkb:, co * 128:(co + 1) * 128])
                        yps = ps_y.tile([128, CHUNK], F32, tag="yps")
                        for p in range(nkb):
                            nc.tensor.matmul(yps[:], v_co[:, nkb - 1 - p, :],
                                             et[:, p, :],
                                             start=(p == 0), stop=(p == nkb - 1))
                        nc.vector.tensor_copy(y_sb[:, co, :], yps[:])

                    # --- out-proj + normalize ---
                    for co in range(CO):
                        zps = ps_z.tile([128, CHUNK], F32, tag="zps")
                        for ci in range(CI):
                            nc.tensor.matmul(
                                zps[:], wo_sb[:, ci, co * 128:(co + 1) * 128],
                                y_sb[:, ci, :], start=(ci == 0), stop=(ci == CI - 1))
                        zst = zstp.tile([128, CHUNK], F32, tag="zst")
                        nc.vector.tensor_tensor(zst[:], zps[:], rb_sb[:],
                                                mybir.AluOpType.mult)
                        nc.sync.dma_start(zt[co * 128:(co + 1) * 128, rsl], zst[:])
    nc.compile()
    _NC_CACHE["nc"] = nc
    return nc


def make_in_maps(inputs):
    x = np.asarray(inputs["x"], dtype=np.float32)
    for bname in ("bq", "bk", "bv", "bo"):
        bval = np.asarray(inputs[bname])
        assert np.all(bval == 0.0), f"{bname} nonzero: unsupported fast path"
    wqt = np.ascontiguousarray(np.asarray(inputs["Wq"], np.float32).T)
    wkt = np.ascontiguousarray(np.asarray(inputs["Wk"], np.float32).T)
    wvt = np.ascontiguousarray(np.asarray(inputs["Wv"], np.float32).T)
    wot = np.ascontiguousarray(np.asarray(inputs["Wo"], np.float32).T)
    keyidx16 = (np.arange(T, dtype=np.float32).reshape(KB, 128).T + WINDOW
                ).copy()  # [128, KB]
    chunk_map = {0: (0, 3), 1: (1, 2)}  # slot 0 = denser chunk
    in_maps = []
    for core in range(N_CORES):
        b, h = divmod(core, 2)
        xt_b = np.ascontiguousarray(x[b].T)  # [C, T]
        ch0, ch1 = chunk_map[h]
        xtq = np.concatenate(
            [xt_b[:, ch0 * CHUNK:(ch0 + 1) * CHUNK],
             xt_b[:, ch1 * CHUNK:(ch1 + 1) * CHUNK]], axis=1)
        rowidx = np.concatenate(
            [np.arange(ch0 * CHUNK, (ch0 + 1) * CHUNK, dtype=np.float32),
             np.arange(ch1 * CHUNK, (ch1 + 1) * CHUNK, dtype=np.float32)])
        rowidxb = np.ascontiguousarray(
            np.broadcast_to(rowidx[None, :], (128, TOWN)))
        in_maps.append({
            "xt": xt_b, "xtq": np.ascontiguousarray(xtq),
            "wqt": wqt, "wkt": wkt, "wvt": wvt, "wot": wot,
            "keyidx16": keyidx16, "rowidxb": rowidxb,
        })
    return in_maps


def gather_output(results, dtype):
    out = np.empty((B, T, C), dtype=dtype)
    chunk_map = {0: (0, 3), 1: (1, 2)}
    for core in range(N_CORES):
        b, h = divmod(core, 2)
        y = results[core]["zt"].T  # [TOWN rows, C]
        ch0, ch1 = chunk_map[h]
        out[b, ch0 * CHUNK:(ch0 + 1) * CHUNK] = y[:CHUNK]
        out[b, ch1 * CHUNK:(ch1 + 1) * CHUNK] = y[CHUNK:]
    return out


def kernel(**inputs):
    nc = build()
    in_maps = make_in_maps(inputs)
    res = bass_utils.run_bass_kernel_spmd(nc, in_maps,
                                          core_ids=list(range(N_CORES)))
    return gather_output(res.results, np.asarray(inputs["x"]).dtype)



# revision 2
# speedup vs baseline: 1.0099x; 1.0099x over previous
"""TRN2 Bass kernel for nn_LocalAttention (B=4, T=2048, C=1024, window=16).

v4 = v3 (weight-folded, bf16, SBUF-resident X, contiguous DMA) +
  - 256-row attention chunks with even/odd global-chunk interleaving:
    h=0 owns global 256-chunks {0,2,4,6}, h=1 owns {1,3,5,7}. Uniform
    slot sweep counts (16,13,9,5)=43 quarter-sweeps vs (16,9)x2=50 at
    512 granularity: ~12us less PE work. h=0 slots are exact; h=1's
    over-included blocks are zeroed by the is_ge mask.
  - DMA priority ordering: sync queue feeds Q' (xq0 per-ci, then m cols,
    then xq1), scalar carries only xt (keys), gpsimd carries
    ki/ri/w2/xv (needed later).

Formulation (biases are zero):
  S   = X M X^T,  M  = Wq^T Wk / sqrt(C)   (host-side fold)
  out = softmax(S) X W2,  W2 = Wv^T Wo^T   (host-side fold)
"""
import numpy as np
import ml_dtypes

import concourse.bass as bass
import concourse.mybir as mybir
import concourse.tile as tile
from concourse import bacc
from concourse import bass_utils

N_CORES = 8
B, T, C = 4, 2048, 1024
WINDOW = 16
TOWN = T // 2          # own rows per core
CHUNK = 256            # rows per attention chunk
NCHUNK = TOWN // CHUNK  # 4
QCH = 256              # rows per Q'-projection chunk
NQCH = TOWN // QCH     # 2
CI = C // 128          # 8 contraction blocks
CO = C // 128          # 8 output blocks
KB = T // 128          # 16 key blocks
# pair-hybrid sweep schedule: own-row chunk pairs (cols 0:512 and
# 512:1024) run 512-wide sweeps for key blocks both chunks keep, plus
# 256-wide boundary sweeps (left half only). Tuples: (n_wide, n_extra,
# masked-wide-positions); extras are always masked.
PAIRS = ((13, 3, range(8, 13)), (5, 4, range(0, 5)))
PCH = 2 * CHUNK  # 512 pair width
F32 = mybir.dt.float32
F32R = mybir.dt.float32r
BF16 = mybir.dt.bfloat16
NP_BF16 = ml_dtypes.bfloat16

_NC_CACHE = {}


def build():
    if "nc" in _NC_CACHE:
        return _NC_CACHE["nc"]
    nc = bacc.Bacc("TRN2", target_bir_lowering=False, debug=False,
                   num_devices=N_CORES)
    # all host-side re-laid-out for contiguous per-partition DMA
    xt = nc.dram_tensor("xt", [128, KB, CI, 128], BF16, kind="ExternalInput").ap()
    xv = nc.dram_tensor("xv", [T, C], BF16, kind="ExternalInput").ap()
    xtq = nc.dram_tensor("xtq", [128, NQCH, CI, QCH], BF16,
                         kind="ExternalInput").ap()
    m = nc.dram_tensor("m", [128, CO, CI, 128], BF16, kind="ExternalInput").ap()
    w2 = nc.dram_tensor("w2", [128, CO, CI, 128], BF16, kind="ExternalInput").ap()
    keyidx16 = nc.dram_tensor("keyidx16", [128, KB], F32, kind="ExternalInput").ap()
    rowidxb = nc.dram_tensor("rowidxb", [128, TOWN], F32, kind="ExternalInput").ap()
    zt = nc.dram_tensor("zt", [C, TOWN], BF16, kind="ExternalOutput").ap()

    xv3 = xv.rearrange("(kb ki) c -> ki kb c", ki=128)

    with tile.TileContext(nc) as tc:
        with tc.tile_pool(name="res", bufs=1) as res, \
             tc.tile_pool(name="etp", bufs=2) as etp, \
             tc.tile_pool(name="ysb", bufs=2) as ysb_p, \
             tc.tile_pool(name="wb", bufs=2) as wb, \
             tc.tile_pool(name="zst", bufs=3) as zstp, \
             tc.tile_pool(name="ps_s", bufs=3, space="PSUM") as ps_s, \
             tc.tile_pool(name="ps_sh", bufs=1, space="PSUM") as ps_sh, \
             tc.tile_pool(name="ps_y", bufs=2, space="PSUM") as ps_y, \
             tc.tile_pool(name="ps_z", bufs=2, space="PSUM") as ps_z:
            xt_sb = res.tile([128, KB, CI, 128], BF16, tag="xt")
            xv_sb = res.tile([128, KB, C], BF16, tag="xv")
            qt_sb = res.tile([128, CI, TOWN], BF16, tag="qt")
            w2_sb = res.tile([128, CO, CI, 128], BF16, tag="w2")
            ki16_sb = res.tile([128, KB], F32, tag="ki16")
            ri_sb = res.tile([128, TOWN], F32, tag="ri")
            ones_row_f32 = res.tile([1, 128], F32, tag="onesrf")
            nc.vector.memset(ones_row_f32[:], 1.0)
            ones_1x128 = res.tile([1, 128], F32R, tag="o1")
            nc.vector.tensor_copy(ones_1x128[:], ones_row_f32[:])
            ones_col_f32 = res.tile([128, 1], F32, tag="onescf")
            nc.vector.memset(ones_col_f32[:], 1.0)
            ones_128x1 = res.tile([128, 1], F32R, tag="o2")
            nc.vector.tensor_copy(ones_128x1[:], ones_col_f32[:])

            # ============ Phase A: Q' projection (Q'^T = M^T X_own^T) =======
            with tc.tile_pool(name="pa", bufs=1) as pa:
                m_sb = pa.tile([128, CO, CI, 128], BF16, tag="m")
                # critical-first on sync: xq0 (per-ci) + m col0 unblock the
                # first matmul group; xq1 only needed ~15us later
                xq_sb0 = pa.tile([128, CI, QCH], BF16, tag="xq", bufs=NQCH)
                # queue speeds (measured): gpsimd ~150GB/s, scalar ~95,
                # sync ~40-85. Critical Q' feed: m on gpsimd, xq on scalar.
                # sync gets late-needed small/low-priority loads.
                nc.scalar.dma_start(xq_sb0[:], xtq[:, 0])
                xq_sbs = [xq_sb0]
                for qch in range(1, NQCH):
                    xq_sbn = pa.tile([128, CI, QCH], BF16, tag="xq", bufs=NQCH)
                    nc.scalar.dma_start(xq_sbn[:], xtq[:, qch])
                    xq_sbs.append(xq_sbn)
                for co in range(CO):
                    nc.gpsimd.dma_start(m_sb[:, co], m[:, co])
                for kb in reversed(range(KB // 2, KB)):
                    nc.scalar.dma_start(xt_sb[:, kb], xt[:, kb])
                nc.sync.dma_start(ki16_sb[:], keyidx16[:])
                nc.sync.dma_start(ri_sb[:], rowidxb[:])
                for kb in reversed(range(KB // 2)):
                    nc.sync.dma_start(xt_sb[:, kb], xt[:, kb])
                for kb in reversed(range(KB - 9, KB)):   # U slot0 needs 15..7
                    nc.gpsimd.dma_start(xv_sb[:, kb, :], xv3[:, kb, :])
                for co in range(CO):
                    nc.gpsimd.dma_start(w2_sb[:, co], w2[:, co])
                for kb in reversed(range(KB - 9)):
                    nc.gpsimd.dma_start(xv_sb[:, kb, :], xv3[:, kb, :])

                for qch in range(NQCH):
                    for co in range(CO):
                        qps = ps_z.tile([128, QCH], F32, tag="zps")
                        for ci in range(CI):
                            nc.tensor.matmul(
                                qps[:], m_sb[:, co, ci, :],
                                xq_sbs[qch][:, ci, :],
                                start=(ci == 0), stop=(ci == CI - 1))
                        nc.vector.tensor_copy(
                            qt_sb[:, co, qch * QCH:(qch + 1) * QCH], qps[:])

            # ============ Phase B: attention + out-proj =====================
            for pr, (n_wide, n_extra, masked_wide) in enumerate(PAIRS):
                rsl = slice(pr * PCH, (pr + 1) * PCH)
                rsl_l = slice(pr * PCH, pr * PCH + CHUNK)  # left (even) half

                et = etp.tile([128, KB, PCH], BF16, tag="et")
                acc = wb.tile([128, PCH], F32R, tag="acc", bufs=2)
                masked = set(masked_wide)
                # --- scores + exp (+ mask) + lane sums, descending kb;
                #     first n_wide sweeps are 512-wide, extras 256-wide ---
                for p in range(n_wide + n_extra):
                    kb = KB - 1 - p
                    wide = p < n_wide
                    csl = slice(0, PCH if wide else CHUNK)
                    mrsl = rsl if wide else rsl_l
                    sps = ps_s.tile([128, PCH], F32, tag="sps")
                    for ci in range(CI):
                        nc.tensor.matmul(
                            sps[:, csl], xt_sb[:, kb, ci, :],
                            qt_sb[:, ci, mrsl],
                            start=(ci == 0), stop=(ci == CI - 1))
                    nc.scalar.activation(et[:, p, csl], sps[:, csl],
                                         mybir.ActivationFunctionType.Exp,
                                         scale=1.0)
                    if (p in masked) or not wide:
                        mask = wb.tile([128, PCH], BF16, tag="mask")
                        nc.vector.tensor_tensor(
                            mask[:, csl],
                            ki16_sb[:, kb:kb + 1].to_broadcast(
                                (128, csl.stop)),
                            ri_sb[:, mrsl], mybir.AluOpType.is_ge)
                        nc.vector.tensor_tensor(et[:, p, csl], et[:, p, csl],
                                                mask[:, csl],
                                                mybir.AluOpType.mult)
                    with nc.allow_low_precision(reason="f32r lane sums"):
                        if p == 0:
                            nc.vector.tensor_copy(acc[:], et[:, p, :])
                        else:
                            nc.vector.tensor_tensor(acc[:, csl], acc[:, csl],
                                                    et[:, p, csl],
                                                    mybir.AluOpType.add)
                # --- single partition-sum matmul over lane sums ---
                sums_ps = ps_sh.tile([1, PCH], F32, tag="shared")
                nc.tensor.matmul(sums_ps[:], ones_128x1[:], acc[:],
                                 start=True, stop=True)
                recip = wb.tile([1, PCH], F32R, tag="recip")
                with nc.allow_low_precision(reason="fp32r normalizer bcast"):
                    nc.vector.reciprocal(recip[:], sums_ps[:])

                # --- U^T = X_blk^T @ E^T per cout block ---
                y_sb = ysb_p.tile([128, CO, PCH], BF16, tag="ysb")
                for co in range(CO):
                    # uniform-width accumulation groups only: wide sweeps in
                    # yps, boundary (left-half) sweeps in a second tile,
                    # combined during the y-copy on DVE
                    yps = ps_y.tile([128, PCH], F32, tag="yps")
                    for p in range(n_wide):
                        nc.tensor.matmul(
                            yps[:],
                            xv_sb[:, KB - 1 - p, co * 128:(co + 1) * 128],
                            et[:, p, :],
                            start=(p == 0), stop=(p == n_wide - 1))
                    yps2 = ps_s.tile([128, CHUNK], F32, tag="sps")
                    for j in range(n_extra):
                        p = n_wide + j
                        nc.tensor.matmul(
                            yps2[:],
                            xv_sb[:, KB - 1 - p, co * 128:(co + 1) * 128],
                            et[:, p, 0:CHUNK],
                            start=(j == 0), stop=(j == n_extra - 1))
                    y2_sb = wb.tile([128, CHUNK], F32, tag="y2sb")
                    nc.vector.tensor_copy(y2_sb[:], yps2[:])
                    nc.vector.tensor_tensor(y_sb[:, co, 0:CHUNK],
                                            yps[:, 0:CHUNK], y2_sb[:],
                                            mybir.AluOpType.add)
                    nc.vector.tensor_copy(y_sb[:, co, CHUNK:PCH],
                                          yps[:, CHUNK:PCH])

                # --- out-proj + normalize (normalizer broadcast after the
                #     first Z group so recip never stalls the PE) ---
                rb_sb = wb.tile([128, PCH], F32, tag="rbsb")
                for co in range(CO):
                    zps = ps_z.tile([128, PCH], F32, tag="zps")
                    for ci in range(CI):
                        nc.tensor.matmul(
                            zps[:], w2_sb[:, co, ci, :],
                            y_sb[:, ci, :], start=(ci == 0), stop=(ci == CI - 1))
                    if co == 0:
                        rb_ps = ps_sh.tile([128, PCH], F32, tag="shared")
                        nc.tensor.matmul(rb_ps[:], ones_1x128[:], recip[:],
                                         start=True, stop=True)
                        nc.vector.tensor_copy(rb_sb[:], rb_ps[:])
                    zst = zstp.tile([128, PCH], BF16, tag="zst")
                    nc.vector.tensor_tensor(zst[:], zps[:], rb_sb[:],
                                            mybir.AluOpType.mult)
                    nc.scalar.dma_start(zt[co * 128:(co + 1) * 128, rsl], zst[:])
    nc.compile()
    _NC_CACHE["nc"] = nc
    return nc


def _to_4d_blocks(a2d):
    """[C, N] -> [128, N//128 outer, C//128 inner, 128-fine] host re-layout
    so each per-outer-block DMA is contiguous per partition."""
    Cdim, N = a2d.shape
    return np.ascontiguousarray(
        a2d.reshape(Cdim // 128, 128, N // 128, 128).transpose(1, 2, 0, 3))


# even/odd interleave: core h owns global 256-chunks [h, h+2, h+4, h+6]
def _own_chunks(h):
    return [h + 2 * i for i in range(NCHUNK)]


def make_in_maps(inputs):
    x = np.asarray(inputs["x"], dtype=np.float32)
    for bname in ("bq", "bk", "bv", "bo"):
        bval = np.asarray(inputs[bname])
        assert np.all(bval == 0.0), f"{bname} nonzero: unsupported fast path"
    wq = np.asarray(inputs["Wq"], np.float32)
    wk = np.asarray(inputs["Wk"], np.float32)
    wv = np.asarray(inputs["Wv"], np.float32)
    wo = np.asarray(inputs["Wo"], np.float32)
    m_full = (wq.T @ wk) * np.float32(1.0 / np.sqrt(C))
    w2_full = wv.T @ wo.T
    m_bf = _to_4d_blocks(m_full.astype(NP_BF16))        # [128, CO, CI, 128]
    w2_bf = _to_4d_blocks(w2_full.astype(NP_BF16))      # [128, CO, CI, 128]
    keyidx16 = (np.arange(T, dtype=np.float32).reshape(KB, 128).T + WINDOW
                ).copy()  # [128, KB]
    in_maps = []
    for core in range(N_CORES):
        b, h = divmod(core, 2)
        xb_bf = x[b].astype(NP_BF16)               # [T, C]
        xt_b = np.ascontiguousarray(xb_bf.T)       # [C, T]
        own = _own_chunks(h)
        xtq = np.concatenate(
            [xt_b[:, g * CHUNK:(g + 1) * CHUNK] for g in own], axis=1)
        rowidx = np.concatenate(
            [np.arange(g * CHUNK, (g + 1) * CHUNK, dtype=np.float32)
             for g in own])
        rowidxb = np.ascontiguousarray(
            np.broadcast_to(rowidx[None, :], (128, TOWN)))
        # xtq 4D blocks: [C, TOWN] -> [128, NQCH(512-wide), CI, 512]
        xtq4 = np.ascontiguousarray(
            xtq.reshape(CI, 128, NQCH, QCH).transpose(1, 2, 0, 3))
        in_maps.append({
            "xt": _to_4d_blocks(xt_b),             # [128, KB, CI, 128]
            "xv": np.ascontiguousarray(xb_bf),
            "xtq": xtq4,                           # [128, NQCH, CI, QCH]
            "m": m_bf, "w2": w2_bf,
            "keyidx16": keyidx16, "rowidxb": rowidxb,
        })
    return in_maps


def gather_output(results, dtype):
    out = np.empty((B, T, C), dtype=dtype)
    for core in range(N_CORES):
        b, h = divmod(core, 2)
        y = results[core]["zt"].T.astype(dtype)  # [TOWN rows, C]
        for i, g in enumerate(_own_chunks(h)):
            out[b, g * CHUNK:(g + 1) * CHUNK] = y[i * CHUNK:(i + 1) * CHUNK]
    return out


def kernel(**inputs):
    nc = build()
    in_maps = make_in_maps(inputs)
    res = bass_utils.run_bass_kernel_spmd(nc, in_maps,
                                          core_ids=list(range(N_CORES)))
    return gather_output(res.results, np.asarray(inputs["x"]).dtype)


# revision 3
# speedup vs baseline: 1.0156x; 1.0056x over previous
"""TRN2 Bass kernel for nn_LocalAttention (B=4, T=2048, C=1024, window=16).

v4 = v3 (weight-folded, bf16, SBUF-resident X, contiguous DMA) +
  - 256-row attention chunks with even/odd global-chunk interleaving:
    h=0 owns global 256-chunks {0,2,4,6}, h=1 owns {1,3,5,7}. Uniform
    slot sweep counts (16,13,9,5)=43 quarter-sweeps vs (16,9)x2=50 at
    512 granularity: ~12us less PE work. h=0 slots are exact; h=1's
    over-included blocks are zeroed by the is_ge mask.
  - DMA priority ordering: sync queue feeds Q' (xq0 per-ci, then m cols,
    then xq1), scalar carries only xt (keys), gpsimd carries
    ki/ri/w2/xv (needed later).

Formulation (biases are zero):
  S   = X M X^T,  M  = Wq^T Wk / sqrt(C)   (host-side fold)
  out = softmax(S) X W2,  W2 = Wv^T Wo^T   (host-side fold)
"""
import numpy as np
import ml_dtypes

import concourse.bass as bass
import concourse.mybir as mybir
import concourse.tile as tile
from concourse import bacc
from concourse import bass_utils

N_CORES = 8
B, T, C = 4, 2048, 1024
WINDOW = 16
TOWN = T // 2          # own rows per core
CHUNK = 256            # rows per attention chunk
NCHUNK = TOWN // CHUNK  # 4
QCH = 256              # rows per Q'-projection chunk
NQCH = TOWN // QCH     # 2
CI = C // 128          # 8 contraction blocks
CO = C // 128          # 8 output blocks
KB = T // 128          # 16 key blocks
# pair-hybrid sweep schedule: own-row chunk pairs (cols 0:512 and
# 512:1024) run 512-wide sweeps for key blocks both chunks keep, plus
# 256-wide boundary sweeps (left half only). Tuples: (n_wide, n_extra,
# masked-wide-positions); extras are always masked.
PAIRS = ((13, 3, range(8, 13)), (5, 4, range(0, 5)))
PCH = 2 * CHUNK  # 512 pair width
F32 = mybir.dt.float32
F32R = mybir.dt.float32r
BF16 = mybir.dt.bfloat16
NP_BF16 = ml_dtypes.bfloat16

_NC_CACHE = {}


def build():
    if "nc" in _NC_CACHE:
        return _NC_CACHE["nc"]
    nc = bacc.Bacc("TRN2", target_bir_lowering=False, debug=False,
                   num_devices=N_CORES)
    # all host-side re-laid-out for contiguous per-partition DMA
    xt = nc.dram_tensor("xt", [128, KB, CI, 128], BF16, kind="ExternalInput").ap()
    xv = nc.dram_tensor("xv", [T, C], BF16, kind="ExternalInput").ap()
    xtq = nc.dram_tensor("xtq", [128, NQCH, CI, QCH], BF16,
                         kind="ExternalInput").ap()
    m = nc.dram_tensor("m", [128, CO, CI, 128], BF16, kind="ExternalInput").ap()
    w2 = nc.dram_tensor("w2", [128, CO, CI, 128], BF16, kind="ExternalInput").ap()
    keyidx16 = nc.dram_tensor("keyidx16", [128, KB], F32, kind="ExternalInput").ap()
    rowidxb = nc.dram_tensor("rowidxb", [128, TOWN], F32, kind="ExternalInput").ap()
    zt = nc.dram_tensor("zt", [C, TOWN], BF16, kind="ExternalOutput").ap()

    xv3 = xv.rearrange("(kb ki) c -> ki kb c", ki=128)

    with tile.TileContext(nc) as tc:
        with tc.tile_pool(name="res", bufs=1) as res, \
             tc.tile_pool(name="etp", bufs=1) as etp, \
             tc.tile_pool(name="ysb", bufs=2) as ysb_p, \
             tc.tile_pool(name="wb", bufs=2) as wb, \
             tc.tile_pool(name="zst", bufs=3) as zstp, \
             tc.tile_pool(name="ps_s", bufs=3, space="PSUM") as ps_s, \
             tc.tile_pool(name="ps_sh", bufs=1, space="PSUM") as ps_sh, \
             tc.tile_pool(name="ps_y", bufs=2, space="PSUM") as ps_y, \
             tc.tile_pool(name="ps_z", bufs=2, space="PSUM") as ps_z:
            xt_sb = res.tile([128, KB, CI, 128], BF16, tag="xt")
            xv_sb = res.tile([128, KB, C], BF16, tag="xv")
            qt_sb = res.tile([128, CI, TOWN], BF16, tag="qt")
            w2_sb = res.tile([128, CO, CI, 128], BF16, tag="w2")
            ki16_sb = res.tile([128, KB], F32, tag="ki16")
            ri_sb = res.tile([128, TOWN], F32, tag="ri")
            ones_row_f32 = res.tile([1, 128], F32, tag="onesrf")
            nc.vector.memset(ones_row_f32[:], 1.0)
            ones_1x128 = res.tile([1, 128], F32R, tag="o1")
            nc.vector.tensor_copy(ones_1x128[:], ones_row_f32[:])
            ones_col_f32 = res.tile([128, 1], F32, tag="onescf")
            nc.vector.memset(ones_col_f32[:], 1.0)
            ones_128x1 = res.tile([128, 1], F32R, tag="o2")
            nc.vector.tensor_copy(ones_128x1[:], ones_col_f32[:])

            # ============ Phase A: Q' projection (Q'^T = M^T X_own^T) =======
            with tc.tile_pool(name="pa", bufs=1) as pa:
                m_sb = pa.tile([128, CO, CI, 128], BF16, tag="m")
                # critical-first on sync: xq0 (per-ci) + m col0 unblock the
                # first matmul group; xq1 only needed ~15us later
                xq_sb0 = pa.tile([128, CI, QCH], BF16, tag="xq", bufs=NQCH)
                # queue speeds (measured): gpsimd ~150GB/s, scalar ~95,
                # sync ~40-85. Critical Q' feed: m on gpsimd, xq on scalar.
                # sync gets late-needed small/low-priority loads.
                nc.scalar.dma_start(xq_sb0[:], xtq[:, 0])
                xq_sbs = [xq_sb0]
                for qch in range(1, NQCH):
                    xq_sbn = pa.tile([128, CI, QCH], BF16, tag="xq", bufs=NQCH)
                    nc.scalar.dma_start(xq_sbn[:], xtq[:, qch])
                    xq_sbs.append(xq_sbn)
                for co in range(CO):
                    nc.gpsimd.dma_start(m_sb[:, co], m[:, co])
                for kb in reversed(range(KB // 2, KB)):
                    nc.scalar.dma_start(xt_sb[:, kb], xt[:, kb])
                nc.sync.dma_start(ki16_sb[:], keyidx16[:])
                nc.sync.dma_start(ri_sb[:], rowidxb[:])
                for kb in reversed(range(KB // 2)):
                    nc.sync.dma_start(xt_sb[:, kb], xt[:, kb])
                for kb in reversed(range(KB - 9, KB)):   # U slot0 needs 15..7
                    nc.gpsimd.dma_start(xv_sb[:, kb, :], xv3[:, kb, :])
                for co in range(CO):
                    nc.gpsimd.dma_start(w2_sb[:, co], w2[:, co])
                for kb in reversed(range(KB - 9)):
                    nc.gpsimd.dma_start(xv_sb[:, kb, :], xv3[:, kb, :])

                for qch in range(NQCH):
                    for co in range(CO):
                        qps = ps_z.tile([128, QCH], F32, tag="zps")
                        for ci in range(CI):
                            nc.tensor.matmul(
                                qps[:], m_sb[:, co, ci, :],
                                xq_sbs[qch][:, ci, :],
                                start=(ci == 0), stop=(ci == CI - 1))
                        nc.vector.tensor_copy(
                            qt_sb[:, co, qch * QCH:(qch + 1) * QCH], qps[:])

            # masks precomputed on DVE during Q' (idle time): one per
            # (pair, masked sweep), so phase-B sweeps cost 2 DVE ops not 3
            mask_map = {}
            masks_sb = res.tile([128, 17, 2 * CHUNK], BF16, tag="masks")
            for pr, (n_wide, n_extra, masked_wide) in enumerate(PAIRS):
                for p in list(masked_wide) + list(range(n_wide,
                                                        n_wide + n_extra)):
                    kb = KB - 1 - p
                    w = 2 * CHUNK if p < n_wide else CHUNK
                    mo = pr * 2 * CHUNK
                    nc.vector.tensor_tensor(
                        masks_sb[:, len(mask_map), 0:w],
                        ki16_sb[:, kb:kb + 1].to_broadcast((128, w)),
                        ri_sb[:, mo:mo + w], mybir.AluOpType.is_ge)
                    mask_map[(pr, p)] = len(mask_map)

            # ============ Phase B: attention + out-proj =====================
            for pr, (n_wide, n_extra, masked_wide) in enumerate(PAIRS):
                rsl = slice(pr * PCH, (pr + 1) * PCH)
                rsl_l = slice(pr * PCH, pr * PCH + CHUNK)  # left (even) half

                et = etp.tile([128, KB, PCH], BF16, tag="et")
                acc = wb.tile([128, PCH], F32R, tag="acc", bufs=2)
                masked = set(masked_wide)
                # --- scores + exp (+ mask) + lane sums, descending kb;
                #     first n_wide sweeps are 512-wide, extras 256-wide ---
                for p in range(n_wide + n_extra):
                    kb = KB - 1 - p
                    wide = p < n_wide
                    csl = slice(0, PCH if wide else CHUNK)
                    mrsl = rsl if wide else rsl_l
                    sps = ps_s.tile([128, PCH], F32, tag="sps")
                    for ci in range(CI):
                        nc.tensor.matmul(
                            sps[:, csl], xt_sb[:, kb, ci, :],
                            qt_sb[:, ci, mrsl],
                            start=(ci == 0), stop=(ci == CI - 1))
                    nc.scalar.activation(et[:, p, csl], sps[:, csl],
                                         mybir.ActivationFunctionType.Exp,
                                         scale=1.0)
                    if (pr, p) in mask_map:
                        nc.vector.tensor_tensor(
                            et[:, p, csl], et[:, p, csl],
                            masks_sb[:, mask_map[(pr, p)], csl],
                            mybir.AluOpType.mult)
                    with nc.allow_low_precision(reason="f32r lane sums"):
                        if p == 0:
                            nc.vector.tensor_copy(acc[:], et[:, p, :])
                        else:
                            nc.vector.tensor_tensor(acc[:, csl], acc[:, csl],
                                                    et[:, p, csl],
                                                    mybir.AluOpType.add)
                # --- single partition-sum matmul over lane sums ---
                sums_ps = ps_sh.tile([1, PCH], F32, tag="shared")
                nc.tensor.matmul(sums_ps[:], ones_128x1[:], acc[:],
                                 start=True, stop=True)
                recip = wb.tile([1, PCH], F32R, tag="recip")
                with nc.allow_low_precision(reason="fp32r normalizer bcast"):
                    nc.vector.reciprocal(recip[:], sums_ps[:])

                # --- U^T = X_blk^T @ E^T per cout block ---
                y_sb = ysb_p.tile([128, CO, PCH], BF16, tag="ysb")
                for co in range(CO):
                    # uniform-width accumulation groups only: wide sweeps in
                    # yps, boundary (left-half) sweeps in a second tile,
                    # combined during the y-copy on DVE
                    yps = ps_y.tile([128, PCH], F32, tag="yps")
                    for p in range(n_wide):
                        nc.tensor.matmul(
                            yps[:],
                            xv_sb[:, KB - 1 - p, co * 128:(co + 1) * 128],
                            et[:, p, :],
                            start=(p == 0), stop=(p == n_wide - 1))
                    yps2 = ps_s.tile([128, CHUNK], F32, tag="sps")
                    for j in range(n_extra):
                        p = n_wide + j
                        nc.tensor.matmul(
                            yps2[:],
                            xv_sb[:, KB - 1 - p, co * 128:(co + 1) * 128],
                            et[:, p, 0:CHUNK],
                            start=(j == 0), stop=(j == n_extra - 1))
                    y2_sb = wb.tile([128, CHUNK], F32, tag="y2sb")
                    nc.vector.tensor_copy(y2_sb[:], yps2[:])
                    nc.vector.tensor_tensor(y_sb[:, co, 0:CHUNK],
                                            yps[:, 0:CHUNK], y2_sb[:],
                                            mybir.AluOpType.add)
                    nc.vector.tensor_copy(y_sb[:, co, CHUNK:PCH],
                                          yps[:, CHUNK:PCH])

                # --- out-proj + normalize (normalizer broadcast after the
                #     first Z group so recip never stalls the PE) ---
                rb_sb = wb.tile([128, PCH], F32, tag="rbsb")
                for co in range(CO):
                    zps = ps_z.tile([128, PCH], F32, tag="zps")
                    for ci in range(CI):
                        nc.tensor.matmul(
                            zps[:], w2_sb[:, co, ci, :],
                            y_sb[:, ci, :], start=(ci == 0), stop=(ci == CI - 1))
                    if co == 0:
                        rb_ps = ps_sh.tile([128, PCH], F32, tag="shared")
                        nc.tensor.matmul(rb_ps[:], ones_1x128[:], recip[:],
                                         start=True, stop=True)
                        nc.vector.tensor_copy(rb_sb[:], rb_ps[:])
                    zst = zstp.tile([128, PCH], BF16, tag="zst")
                    nc.vector.tensor_tensor(zst[:], zps[:], rb_sb[:],
                                            mybir.AluOpType.mult)
                    zeng = nc.scalar if co % 2 == 0 else nc.sync
                    zeng.dma_start(zt[co * 128:(co + 1) * 128, rsl], zst[:])
    nc.compile()
    _NC_CACHE["nc"] = nc
    return nc


def _to_4d_blocks(a2d):
    """[C, N] -> [128, N//128 outer, C//128 inner, 128-fine] host re-layout
    so each per-outer-block DMA is contiguous per partition."""
    Cdim, N = a2d.shape
    return np.ascontiguousarray(
        a2d.reshape(Cdim // 128, 128, N // 128, 128).transpose(1, 2, 0, 3))


# even/odd interleave: core h owns global 256-chunks [h, h+2, h+4, h+6]
def _own_chunks(h):
    return [h + 2 * i for i in range(NCHUNK)]


def make_in_maps(inputs):
    x = np.asarray(inputs["x"], dtype=np.float32)
    for bname in ("bq", "bk", "bv", "bo"):
        bval = np.asarray(inputs[bname])
        assert np.all(bval == 0.0), f"{bname} nonzero: unsupported fast path"
    wq = np.asarray(inputs["Wq"], np.float32)
    wk = np.asarray(inputs["Wk"], np.float32)
    wv = np.asarray(inputs["Wv"], np.float32)
    wo = np.asarray(inputs["Wo"], np.float32)
    m_full = (wq.T @ wk) * np.float32(1.0 / np.sqrt(C))
    w2_full = wv.T @ wo.T
    m_bf = _to_4d_blocks(m_full.astype(NP_BF16))        # [128, CO, CI, 128]
    w2_bf = _to_4d_blocks(w2_full.astype(NP_BF16))      # [128, CO, CI, 128]
    keyidx16 = (np.arange(T, dtype=np.float32).reshape(KB, 128).T + WINDOW
                ).copy()  # [128, KB]
    in_maps = []
    for core in range(N_CORES):
        b, h = divmod(core, 2)
        xb_bf = x[b].astype(NP_BF16)               # [T, C]
        xt_b = np.ascontiguousarray(xb_bf.T)       # [C, T]
        own = _own_chunks(h)
        xtq = np.concatenate(
            [xt_b[:, g * CHUNK:(g + 1) * CHUNK] for g in own], axis=1)
        rowidx = np.concatenate(
            [np.arange(g * CHUNK, (g + 1) * CHUNK, dtype=np.float32)
             for g in own])
        rowidxb = np.ascontiguousarray(
            np.broadcast_to(rowidx[None, :], (128, TOWN)))
        # xtq 4D blocks: [C, TOWN] -> [128, NQCH(512-wide), CI, 512]
        xtq4 = np.ascontiguousarray(
            xtq.reshape(CI, 128, NQCH, QCH).transpose(1, 2, 0, 3))
        in_maps.append({
            "xt": _to_4d_blocks(xt_b),             # [128, KB, CI, 128]
            "xv": np.ascontiguousarray(xb_bf),
            "xtq": xtq4,                           # [128, NQCH, CI, QCH]
            "m": m_bf, "w2": w2_bf,
            "keyidx16": keyidx16, "rowidxb": rowidxb,
        })
    return in_maps


def gather_output(results, dtype):
    out = np.empty((B, T, C), dtype=dtype)
    for core in range(N_CORES):
        b, h = divmod(core, 2)
        y = results[core]["zt"].T.astype(dtype)  # [TOWN rows, C]
        for i, g in enumerate(_own_chunks(h)):
            out[b, g * CHUNK:(g + 1) * CHUNK] = y[i * CHUNK:(i + 1) * CHUNK]
    return out


def kernel(**inputs):
    nc = build()
    in_maps = make_in_maps(inputs)
    res = bass_utils.run_bass_kernel_spmd(nc, in_maps,
                                          core_ids=list(range(N_CORES)))
    return gather_output(res.results, np.asarray(inputs["x"]).dtype)
